# revision 16
# baseline (speedup 1.0000x reference)
"""Trainium2 Bass kernel for nn_DRNN_75204877353433.

Windowed bidirectional GRU (W=15) over [B=32, S=512] token ids ->
batch-norm (training stats over B,S) -> MLP -> masked max-pool -> linear.

Strategy (8 NeuronCores, data-parallel over batch):
  - each core handles BC=4 batch rows (2048 window positions), runs the
    full windowed GRU scan for both directions;
  - embedding table is pruned per core on host (only rows the core's
    tokens index, a sharding of the table); the gather itself runs on
    device via indirect DMA;
  - per-token input projections xg = W_ih @ e(tok) + biases are computed
    once per direction on the PE (15x reuse across window positions);
  - recurrent steps: hg = W_hh @ h accumulated in PSUM, with the xr/xz
    additions folded in as identity matmuls; sigmoids/tanh on the scalar
    engine read PSUM directly; the n-gate r*hn and the h-update run on
    the vector engine in bf16;
  - batch-norm statistics: fused multiply+reduce passes, 4KB AllReduce
    across the 8 cores;
  - mask is folded through the MLP algebraically; the -65500 pool shift
    is folded into the final linear bias on host.
"""

import sys

for _p in ("/opt/trn_rl_repo",):
    if _p not in sys.path:
        sys.path.insert(0, _p)

import numpy as np
import ml_dtypes

from concourse import bacc, mybir, tile
from concourse.bass import IndirectOffsetOnAxis
from concourse.bass_utils import run_bass_kernel_spmd

F32 = mybir.dt.float32
BF16 = mybir.dt.float16    # 16-bit compute dtype: fp16 (10-bit mantissa)
I32 = mybir.dt.int32
AF = mybir.ActivationFunctionType
OP = mybir.AluOpType
AX = mybir.AxisListType


class Cfg:
    def __init__(self, B=32, S=512, W=15, E=300, H=256, C=2, n_cores=8,
                 use_cc=True, use_gather=True, use_ttr=False):
        self.B, self.S, self.W, self.E, self.H, self.C = B, S, W, E, H, C
        self.n_cores = n_cores
        self.use_cc = use_cc
        self.use_gather = use_gather
        self.use_ttr = use_ttr
        self.G = 3 * H
        self.BC = B // n_cores                      # batch rows per core
        seg = S + 2 * (W - 1)                       # valid token cols per row
        self.SEG = ((self.BC * seg + 127) // 128 * 128) // self.BC \
            if (self.BC * seg) % 128 else seg
        # pad per-row segment so BC*SEG is a multiple of 128
        while (self.BC * self.SEG) % 128:
            self.SEG += 1
        assert self.SEG >= seg
        self.TC = self.BC * self.SEG                # token cols per core
        self.NT = self.TC // 128                    # gather tiles
        self.NR = self.BC * S                       # window rows per core
        self.HK = (H + 127) // 128                  # H partition tiles (2)
        self.GS = self.G // 128                     # G subtiles (6)
        self.EK = [(k * 128, min(128, E - k * 128))
                   for k in range((E + 127) // 128)]
        self.CHT = (2 * H) // 128                   # hidden channel tiles (4)
        # xg column chunks for the precompute matmuls
        self.XCH = [(i * 512, min(512, self.TC - i * 512))
                    for i in range((self.TC + 511) // 512)]
        assert H % 128 == 0 and self.G % 128 == 0


def build(cfg: Cfg):
    """Build + bacc-compile the Bass program. Returns (nc, out_name)."""
    nc = bacc.Bacc("TRN2", target_bir_lowering=False, debug=False,
                   enable_asserts=False, num_devices=cfg.n_cores)
    # register the BN epsilon as a const AP (0.0/1.0 are pre-registered)
    _eps_t = nc.alloc_sbuf_tensor("const-eps", [128, 1], F32)
    nc.gpsimd.memset(_eps_t.ap(), 1e-5)
    nc.const_aps.aps[(F32, 1e-5)] = _eps_t.ap()
    nc.all_engine_barrier()
    B, S, W, E, H, C = cfg.B, cfg.S, cfg.W, cfg.E, cfg.H, cfg.C
    BC, SEG, TC, NT, NR, HK, GS = (cfg.BC, cfg.SEG, cfg.TC, cfg.NT, cfg.NR,
                                   cfg.HK, cfg.GS)

    def din(name, shape, dt):
        return nc.dram_tensor(name, shape, dt, kind="ExternalInput").ap()

    ptab = din("ptab", [TC, E], F32)
    ids = din("ids", [TC, 1], I32)
    maskin = din("maskin", [128, NR], F32)
    wih = [din(f"wih{d}", [E, cfg.G], BF16) for d in range(2)]
    whh = [din(f"whh{d}", [H, cfg.G], BF16) for d in range(2)]
    bgd = [din(f"bg{d}", [128, GS], F32) for d in range(2)]
    bhnd = [din(f"bhn{d}", [128, HK], F32) for d in range(2)]
    identf_d = din("identf", [128, 128], F32)
    identb_d = din("identb", [128, 128], BF16)
    bng_d = din("bng", [128, cfg.CHT], F32)
    bnb_d = din("bnb", [128, cfg.CHT], F32)
    mb65_d = din("mb65", [128, cfg.CHT], F32)
    mwt_d = din("mwt", [2 * H, 2 * H], BF16)
    lwt_d = din("lwt", [2 * H, C], F32)
    lb4_d = din("lb4", [BC, C], F32)
    out_d = nc.dram_tensor("out", [BC, C], F32, kind="ExternalOutput").ap()

    inv_n = 1.0 / float(B * S)

    with tile.TileContext(nc) as tc:
        # ---- persistent constants -------------------------------------
        constp = tc.alloc_tile_pool(name="const", bufs=1)
        identf = constp.tile([128, 128], F32)
        identb = constp.tile([128, 128], BF16)
        nc.sync.dma_start(identf[:], identf_d[:])
        nc.sync.dma_start(identb[:], identb_d[:])
        whh_t = [[constp.tile([128, cfg.G], BF16, name=f"whh{d}_{k}")
                  for k in range(HK)] for d in range(2)]
        bg_t = [constp.tile([128, GS], F32, name=f"bg{d}") for d in range(2)]
        bhn_t = [constp.tile([128, HK], F32, name=f"bhn{d}") for d in range(2)]
        for d in range(2):
            for k in range(HK):
                nc.sync.dma_start(whh_t[d][k][:], whh[d][k * 128:(k + 1) * 128, :])
            nc.sync.dma_start(bg_t[d][:], bgd[d][:])
            nc.sync.dma_start(bhn_t[d][:], bhnd[d][:])

        # xg + h + hidden persistent tiles
        xgp = tc.alloc_tile_pool(name="xg", bufs=1)
        xg = [[xgp.tile([128, TC], BF16, name=f"xg{d}_{g}")
               for g in range(GS)] for d in range(2)]
        hp = tc.alloc_tile_pool(name="h", bufs=1)
        h_t = [[hp.tile([128, NR], BF16, name=f"h{d}_{k}") for k in range(HK)]
               for d in range(2)]
        hidp = tc.alloc_tile_pool(name="hid", bufs=1, side="right")
        hid = [hidp.tile([128, NR], BF16, name=f"hid{ct}")
               for ct in range(cfg.CHT)]

        # ---- phase A: gather + transpose + xg precompute ---------------
        with tc.tile_pool(name="wihp", bufs=1) as wihp, \
             tc.tile_pool(name="idsp", bufs=2) as idsp, \
             tc.tile_pool(name="eraw", bufs=3) as erawp, \
             tc.tile_pool(name="eT", bufs=1) as eTp, \
             tc.tile_pool(name="tpsum", bufs=2, space="PSUM") as tpsump, \
             tc.tile_pool(name="xgpsum", bufs=4, space="PSUM") as xgpsump:
            wih_t = [[wihp.tile([128, cfg.G], BF16, name=f"wih{d}_{k}")
                      for k in range(len(cfg.EK))] for d in range(2)]
            for d in range(2):
                for k, (e0, ew) in enumerate(cfg.EK):
                    nc.sync.dma_start(wih_t[d][k][:ew, :], wih[d][e0:e0 + ew, :])
            eT = [eTp.tile([128, TC], BF16, name=f"eT{k}")
                  for k in range(len(cfg.EK))]
            for t in range(NT):
                idt = idsp.tile([128, 1], I32)
                nc.sync.dma_start(idt[:], ids[t * 128:(t + 1) * 128, :])
                er = erawp.tile([128, E], F32)
                if cfg.use_gather:
                    nc.gpsimd.indirect_dma_start(
                        out=er[:], out_offset=None, in_=ptab[:],
                        in_offset=IndirectOffsetOnAxis(ap=idt[:, :1], axis=0),
                    )
                else:
                    nc.sync.dma_start(er[:], ptab[t * 128:(t + 1) * 128, :])
                for k, (e0, ew) in enumerate(cfg.EK):
                    tp = tpsump.tile([128, 128], F32, space="PSUM")
                    nc.tensor.transpose(out=tp[:ew, :], in_=er[:, e0:e0 + ew],
                                        identity=identf[:])
                    nc.vector.tensor_copy(
                        out=eT[k][:ew, t * 128:(t + 1) * 128], in_=tp[:ew, :])
            for d in range(2):
                for g in range(GS):
                    for (c0, cw) in cfg.XCH:
                        p = xgpsump.tile([128, 512], F32, space="PSUM")
                        nk = len(cfg.EK)
                        for k, (e0, ew) in enumerate(cfg.EK):
                            nc.tensor.matmul(
                                p[:, :cw],
                                lhsT=wih_t[d][k][:ew, g * 128:(g + 1) * 128],
                                rhs=eT[k][:ew, c0:c0 + cw],
                                start=(k == 0), stop=(k == nk - 1))
                        nc.scalar.activation(
                            out=xg[d][g][:, c0:c0 + cw], in_=p[:, :cw],
                            func=AF.Identity, bias=bg_t[d][:, g:g + 1])

        # ---- phase B: the windowed GRU scan ----------------------------
        with tc.tile_pool(name="rz", bufs=3) as rzp, \
             tc.tile_pool(name="tg", bufs=3) as tgp, \
             tc.tile_pool(name="tn", bufs=3) as tnp, \
             tc.tile_pool(name="ng", bufs=3) as ngp, \
             tc.tile_pool(name="dg", bufs=3) as dgp, \
             tc.tile_pool(name="eg", bufs=3) as egp, \
             tc.tile_pool(name="prz", bufs=2, space="PSUM") as przp, \
             tc.tile_pool(name="pn", bufs=4, space="PSUM") as pnp:
            for w in range(W):
                last = (w == W - 1)
                for c in range(BC):
                    hc = slice(c * S, (c + 1) * S)
                    for d in range(2):
                        off = w if d == 0 else 2 * (W - 1) - w
                        base = c * SEG + off

                        def xs(g):
                            return xg[d][g][:, base:base + S]

                        if w == 0:
                            # h0 = 0: h1 = (1-z)*n, n = tanh(xn + r*bhh_n)
                            for k in range(HK):
                                r_ = tgp.tile([128, S], BF16, tag="t")
                                nc.scalar.activation(r_[:], xs(k), AF.Sigmoid)
                                z_ = dgp.tile([128, S], BF16, tag="d")
                                nc.scalar.activation(z_[:], xs(HK + k),
                                                     AF.Sigmoid)
                                tn_ = tnp.tile([128, S], BF16, tag="tn")
                                nc.vector.scalar_tensor_tensor(
                                    out=tn_[:], in0=r_[:],
                                    scalar=bhn_t[d][:, k:k + 1],
                                    in1=xs(2 * HK + k),
                                    op0=OP.mult, op1=OP.add)
                                nn_ = ngp.tile([128, S], BF16, tag="n")
                                nc.scalar.activation(nn_[:], tn_[:], AF.Tanh,
                                                     scale=-1.0)
                                nc.vector.scalar_tensor_tensor(
                                    out=h_t[d][k][:, hc], in0=z_[:],
                                    scalar=1.0, in1=nn_[:],
                                    op0=OP.subtract, op1=OP.mult)
                            continue

                        prz = []
                        for k in range(HK):
                            pz = przp.tile([128, 2 * S], F32, space="PSUM",
                                           tag="prz")
                            prz.append(pz)
                            for half, g in ((0, k), (1, HK + k)):
                                dst = pz[:, half * S:(half + 1) * S]
                                for kk in range(HK):
                                    nc.tensor.matmul(
                                        dst,
                                        lhsT=whh_t[d][kk][:, g * 128:(g + 1) * 128],
                                        rhs=h_t[d][kk][:, hc],
                                        start=(kk == 0), stop=False)
                                nc.tensor.matmul(dst, lhsT=identb[:],
                                                 rhs=xs(g),
                                                 start=False, stop=True)
                        pn = []
                        for k in range(HK):
                            pk = pnp.tile([128, S], F32, space="PSUM", tag="pn")
                            pn.append(pk)
                            g = 2 * HK + k
                            for kk in range(HK):
                                nc.tensor.matmul(
                                    pk[:],
                                    lhsT=whh_t[d][kk][:, g * 128:(g + 1) * 128],
                                    rhs=h_t[d][kk][:, hc],
                                    start=(kk == 0), stop=(kk == HK - 1))
                        for k in range(HK):
                            rz_ = rzp.tile([128, 2 * S], BF16, tag="rz")
                            nc.scalar.activation(rz_[:], prz[k][:], AF.Sigmoid)
                            t_ = tgp.tile([128, S], BF16, tag="t")
                            nc.vector.scalar_tensor_tensor(
                                out=t_[:], in0=pn[k][:],
                                scalar=bhn_t[d][:, k:k + 1],
                                in1=rz_[:, 0:S], op0=OP.add, op1=OP.mult)
                            tn_ = tnp.tile([128, S], BF16, tag="tn")
                            nc.vector.tensor_tensor(
                                out=tn_[:], in0=t_[:], in1=xs(2 * HK + k),
                                op=OP.add)
                            n_ = ngp.tile([128, S], BF16, tag="n")
                            nc.scalar.activation(n_[:], tn_[:], AF.Tanh)
                            d_ = dgp.tile([128, S], BF16, tag="d")
                            nc.vector.tensor_tensor(
                                out=d_[:], in0=h_t[d][k][:, hc], in1=n_[:],
                                op=OP.subtract)
                            e_ = egp.tile([128, S], BF16, tag="e")
                            nc.vector.tensor_tensor(
                                out=e_[:], in0=rz_[:, S:2 * S], in1=d_[:],
                                op=OP.mult)
                            dest = (hid[d * HK + k] if last else h_t[d][k])
                            nc.vector.tensor_tensor(
                                out=dest[:, hc], in0=n_[:], in1=e_[:],
                                op=OP.add)

        hp.release()
        xgp.release()

        # ---- phase C: BN stats + AllReduce + affine --------------------
        nrmp = tc.alloc_tile_pool(name="nrm", bufs=1, side="right")
        nrm = [nrmp.tile([128, NR], BF16, name=f"nrm{ct}")
               for ct in range(cfg.CHT)]
        maskp = tc.alloc_tile_pool(name="maskp", bufs=1, side="right")
        mask_t = maskp.tile([128, NR], F32)
        nc.sync.dma_start(mask_t[:], maskin[:])
        with tc.tile_pool(name="scr", bufs=2) as scrp, \
             tc.tile_pool(name="stat", bufs=1) as statp, \
             tc.tile_pool(name="dram", bufs=1, space="DRAM") as dramp:
            sums = statp.tile([128, 2 * cfg.CHT], F32)
            dummy = statp.tile([128, 1], F32)
            for ct in range(cfg.CHT):
                sc = scrp.tile([128, NR], BF16, tag="scr")
                if cfg.use_ttr:
                    nc.vector.tensor_tensor_reduce(
                        out=sc[:], in0=hid[ct][:], in1=mask_t[:],
                        scale=1.0, scalar=0.0, op0=OP.mult, op1=OP.add,
                        accum_out=sums[:, ct:ct + 1])
                    nc.vector.tensor_tensor_reduce(
                        out=dummy.broadcast_to([128, NR]), in0=sc[:],
                        in1=sc[:], scale=1.0, scalar=0.0, op0=OP.mult,
                        op1=OP.add,
                        accum_out=sums[:, cfg.CHT + ct:cfg.CHT + ct + 1])
                else:
                    nc.vector.tensor_tensor(out=sc[:], in0=hid[ct][:],
                                            in1=mask_t[:], op=OP.mult)
                    nc.vector.tensor_reduce(out=sums[:, ct:ct + 1],
                                            in_=sc[:], axis=AX.X, op=OP.add)
                    sq = scrp.tile([128, NR], BF16, tag="scr2")
                    nc.vector.tensor_tensor(out=sq[:], in0=sc[:], in1=sc[:],
                                            op=OP.mult)
                    nc.vector.tensor_reduce(
                        out=sums[:, cfg.CHT + ct:cfg.CHT + ct + 1],
                        in_=sq[:], axis=AX.X, op=OP.add)
            gsums = statp.tile([128, 2 * cfg.CHT], F32)
            if cfg.use_cc:
                bnc_in = dramp.tile([128, 2 * cfg.CHT], F32)
                bnc_out = dramp.tile([128, 2 * cfg.CHT], F32,
                                     addr_space="Shared")
                nc.gpsimd.dma_start(bnc_in[:], sums[:])
                nc.gpsimd.collective_compute(
                    "AllReduce", OP.add,
                    replica_groups=[list(range(cfg.n_cores))],
                    ins=[bnc_in.opt()], outs=[bnc_out.opt()])
                nc.gpsimd.dma_start(gsums[:], bnc_out[:])
            else:
                nc.vector.tensor_copy(out=gsums[:], in_=sums[:])

            bng_t = statp.tile([128, cfg.CHT], F32)
            bnb_t = statp.tile([128, cfg.CHT], F32)
            nc.sync.dma_start(bng_t[:], bng_d[:])
            nc.sync.dma_start(bnb_t[:], bnb_d[:])
            abuf = statp.tile([128, cfg.CHT], F32)
            bbuf = statp.tile([128, cfg.CHT], F32)
            with nc.allow_low_precision("bn 1/sqrt + NR refine"), \
                 tc.tile_pool(name="stt", bufs=2) as sttp:
                for ct in range(cfg.CHT):
                    gs_s = gsums[:, ct:ct + 1]
                    gs_q = gsums[:, cfg.CHT + ct:cfg.CHT + ct + 1]
                    mu = sttp.tile([128, 1], F32, tag="mu")
                    nc.scalar.mul(mu[:], gs_s, inv_n)
                    mq = sttp.tile([128, 1], F32, tag="mq")
                    nc.scalar.square(mq[:], mu[:])
                    varp = sttp.tile([128, 1], F32, tag="var")
                    # var + eps = sumsq/n - mu^2 + eps
                    nc.vector.scalar_tensor_tensor(
                        out=varp[:], in0=gs_q, scalar=inv_n, in1=mq[:],
                        op0=OP.mult, op1=OP.subtract)
                    nc.scalar.add(varp[:], varp[:], 1e-5)  # varp = var + eps
                    sd = sttp.tile([128, 1], F32, tag="sd")
                    nc.scalar.sqrt(sd[:], varp[:])
                    y0 = sttp.tile([128, 1], F32, tag="y0")
                    nc.vector.reciprocal(y0[:], sd[:])
                    # one Newton step: y1 = y0*(1.5 - 0.5*var*y0^2)
                    y2 = sttp.tile([128, 1], F32, tag="y2")
                    nc.vector.tensor_tensor(out=y2[:], in0=y0[:], in1=y0[:],
                                            op=OP.mult)
                    vy2 = sttp.tile([128, 1], F32, tag="vy2")
                    nc.vector.tensor_tensor(out=vy2[:], in0=varp[:], in1=y2[:],
                                            op=OP.mult)
                    nc.vector.tensor_scalar(
                        out=vy2[:], in0=vy2[:], scalar1=-0.5, scalar2=1.5,
                        op0=OP.mult, op1=OP.add)
                    y1 = sttp.tile([128, 1], F32, tag="y1")
                    nc.vector.tensor_tensor(out=y1[:], in0=y0[:], in1=vy2[:],
                                            op=OP.mult)
                    nc.vector.tensor_tensor(out=abuf[:, ct:ct + 1],
                                            in0=bng_t[:, ct:ct + 1],
                                            in1=y1[:], op=OP.mult)
                    mua = sttp.tile([128, 1], F32, tag="mua")
                    nc.vector.tensor_tensor(out=mua[:], in0=mu[:],
                                            in1=abuf[:, ct:ct + 1],
                                            op=OP.mult)
                    nc.vector.tensor_tensor(out=bbuf[:, ct:ct + 1],
                                            in0=bnb_t[:, ct:ct + 1],
                                            in1=mua[:], op=OP.subtract)
            for ct in range(cfg.CHT):
                nc.vector.tensor_scalar(
                    out=nrm[ct][:], in0=hid[ct][:],
                    scalar1=abuf[:, ct:ct + 1], scalar2=bbuf[:, ct:ct + 1],
                    op0=OP.mult, op1=OP.add)
        # ---- phase D: MLP + masked max-pool + linear -------------------
        with tc.tile_pool(name="mwtp", bufs=1) as mwtp, \
             tc.tile_pool(name="tailc", bufs=1) as tailc, \
             tc.tile_pool(name="qp", bufs=3) as qp, \
             tc.tile_pool(name="pmlp", bufs=4, space="PSUM") as pmlpp, \
             tc.tile_pool(name="pfin", bufs=1, space="PSUM") as pfinp:
            mwt_t = [mwtp.tile([128, 2 * H], BF16, name=f"mwt{kt}")
                     for kt in range(cfg.CHT)]
            for kt in range(cfg.CHT):
                nc.sync.dma_start(mwt_t[kt][:], mwt_d[kt * 128:(kt + 1) * 128, :])
            mb65_t = tailc.tile([128, cfg.CHT], F32)
            nc.sync.dma_start(mb65_t[:], mb65_d[:])
            # moff = (mask - 1) * 65500 : 0 at live positions, -65500 at
            # masked ones; added after the mask-multiply so live values keep
            # full fp32 precision (a +65500 shift would quantize them).
            moff_t = tailc.tile([128, NR], F32)
            nc.vector.tensor_scalar(
                out=moff_t[:], in0=mask_t[:], scalar1=1.0, scalar2=65500.0,
                op0=OP.subtract, op1=OP.mult)
            lwt_t = [tailc.tile([128, C], F32, name=f"lwt{kt}")
                     for kt in range(cfg.CHT)]
            for kt in range(cfg.CHT):
                nc.sync.dma_start(lwt_t[kt][:], lwt_d[kt * 128:(kt + 1) * 128, :])
            lb_t = tailc.tile([128, C], F32)
            nc.sync.dma_start(lb_t[:BC, :], lb4_d[:, :])
            pld = [tailc.tile([128, BC], F32, name=f"pld{mt}")
                   for mt in range(cfg.CHT)]
            for c in range(BC):
                hc = slice(c * S, (c + 1) * S)
                for mt in range(cfg.CHT):
                    pm = pmlpp.tile([128, S], F32, space="PSUM", tag="pm")
                    for kt in range(cfg.CHT):
                        nc.tensor.matmul(
                            pm[:],
                            lhsT=mwt_t[kt][:, mt * 128:(mt + 1) * 128],
                            rhs=nrm[kt][:, hc],
                            start=(kt == 0), stop=(kt == cfg.CHT - 1))
                    qa = qp.tile([128, S], F32, tag="qa")
                    nc.vector.scalar_tensor_tensor(
                        out=qa[:], in0=pm[:], scalar=mb65_t[:, mt:mt + 1],
                        in1=mask_t[:, hc], op0=OP.add, op1=OP.mult)
                    q = qp.tile([128, S], F32, tag="q")
                    nc.vector.tensor_tensor(
                        out=q[:], in0=qa[:], in1=moff_t[:, hc], op=OP.add)
                    nc.vector.tensor_reduce(
                        out=pld[mt][:, c:c + 1], in_=q[:], axis=AX.X,
                        op=OP.max)
            pf = pfinp.tile([128, C], F32, space="PSUM")
            for mt in range(cfg.CHT):
                nc.tensor.matmul(pf[:BC, :], lhsT=pld[mt][:, :BC],
                                 rhs=lwt_t[mt][:, :],
                                 start=(mt == 0), stop=(mt == cfg.CHT - 1))
            ob = tailc.tile([128, C], F32)
            nc.vector.tensor_tensor(out=ob[:BC, :], in0=pf[:BC, :],
                                    in1=lb_t[:BC, :], op=OP.add)
            nc.sync.dma_start(out_d[:, :], ob[:BC, :])
        maskp.release()
        nrmp.release()
        hidp.release()
        constp.release()

    nc.compile()
    return nc


def prep_inputs(inputs, cfg: Cfg):
    """Host-side sharding/prep. Returns in_maps (one dict per core)."""
    B, S, W, E, H, C = cfg.B, cfg.S, cfg.W, cfg.E, cfg.H, cfg.C
    x = np.asarray(inputs["x"]).astype(np.int64)
    emb = np.asarray(inputs["emb"], dtype=np.float32)
    mask = (x > 0).astype(np.float32)                       # [B, S]

    def bf(a):
        return np.ascontiguousarray(np.asarray(a, np.float32)
                                    .astype(np.float16))

    def f32(a):
        return np.ascontiguousarray(np.asarray(a, dtype=np.float32))

    shared = {}
    for d, sfx in enumerate("fb"):
        W_ih = np.asarray(inputs[f"W_ih_{sfx}"], np.float32)
        W_hh = np.asarray(inputs[f"W_hh_{sfx}"], np.float32)
        b_ih = np.asarray(inputs[f"b_ih_{sfx}"], np.float32)
        b_hh = np.asarray(inputs[f"b_hh_{sfx}"], np.float32)
        shared[f"wih{d}"] = bf(W_ih.T)                       # [E, G]
        shared[f"whh{d}"] = bf(W_hh.T)                       # [H, G]
        bfold = b_ih.copy()
        bfold[:2 * H] += b_hh[:2 * H]                        # r,z gates
        shared[f"bg{d}"] = f32(bfold.reshape(cfg.GS, 128).T)  # [128, GS]
        shared[f"bhn{d}"] = f32(b_hh[2 * H:].reshape(cfg.HK, 128).T)
    shared["identf"] = f32(np.eye(128))
    shared["identb"] = bf(np.eye(128))
    shared["bng"] = f32(np.asarray(inputs["bn_gamma"], np.float32)
                        .reshape(cfg.CHT, 128).T)
    shared["bnb"] = f32(np.asarray(inputs["bn_beta"], np.float32)
                        .reshape(cfg.CHT, 128).T)
    mlp_b = np.asarray(inputs["mlp_b"], np.float32)
    shared["mb65"] = f32(mlp_b.reshape(cfg.CHT, 128).T)
    shared["mwt"] = bf(np.asarray(inputs["mlp_W"], np.float32).T)
    lin_W = np.asarray(inputs["lin_W"], np.float32)
    lin_b = np.asarray(inputs["lin_b"], np.float32)
    shared["lwt"] = f32(lin_W.T)                             # [2H, C]
    shared["lb4"] = f32(np.broadcast_to(lin_b[None, :], (cfg.BC, C)))

    in_maps = []
    for core in range(cfg.n_cores):
        rows = x[core * cfg.BC:(core + 1) * cfg.BC]          # [BC, S]
        ids = np.zeros((cfg.BC, cfg.SEG), np.int64)
        ids[:, W - 1:W - 1 + S] = rows
        ids = ids.reshape(-1)                                # [TC]
        uids, inv = np.unique(ids, return_inverse=True)
        pt = np.zeros((cfg.TC, E), np.float32)
        if cfg.use_gather:
            pt[:len(uids)] = emb[uids]
        else:
            pt[:] = emb[ids]
        m = {k: v for k, v in shared.items()}
        m["ptab"] = pt
        m["ids"] = np.ascontiguousarray(inv.astype(np.int32)[:, None])
        mrow = mask[core * cfg.BC:(core + 1) * cfg.BC].reshape(-1)  # [NR]
        m["maskin"] = np.ascontiguousarray(
            np.broadcast_to(mrow[None, :], (128, cfg.NR)).astype(np.float32))
        in_maps.append(m)
    return in_maps


_CACHE = {}


def get_compiled(cfg: Cfg | None = None):
    key = "default" if cfg is None else id(cfg)
    if key not in _CACHE:
        _CACHE[key] = build(cfg or Cfg())
    return _CACHE[key]


def kernel(**inputs) -> np.ndarray:
    cfg = Cfg()
    nc = get_compiled(None)
    in_maps = prep_inputs(inputs, cfg)
    res = run_bass_kernel_spmd(nc, in_maps, core_ids=list(range(cfg.n_cores)))
    return np.concatenate([res.results[i]["out"] for i in range(cfg.n_cores)],
                          axis=0).astype(np.float32)


# revision 18
# speedup vs baseline: 1.0416x; 1.0416x over previous
"""Trainium2 Bass kernel for nn_DRNN_75204877353433.

Windowed bidirectional GRU (W=15) over [B=32, S=512] token ids ->
batch-norm (training stats over B,S) -> MLP -> masked max-pool -> linear.

Strategy (8 NeuronCores, data-parallel over batch):
  - each core handles BC=4 batch rows (2048 window positions), runs the
    full windowed GRU scan for both directions;
  - embedding table is pruned per core on host (only rows the core's
    tokens index, a sharding of the table); the gather itself runs on
    device via indirect DMA;
  - per-token input projections xg = W_ih @ e(tok) + biases are computed
    once per direction on the PE (15x reuse across window positions);
  - recurrent steps: hg = W_hh @ h accumulated in PSUM, with the xr/xz
    additions folded in as identity matmuls; sigmoids/tanh on the scalar
    engine read PSUM directly; the n-gate r*hn and the h-update run on
    the vector engine in bf16;
  - batch-norm statistics: fused multiply+reduce passes, 4KB AllReduce
    across the 8 cores;
  - mask is folded through the MLP algebraically; the -65500 pool shift
    is folded into the final linear bias on host.
"""

import sys

for _p in ("/opt/trn_rl_repo",):
    if _p not in sys.path:
        sys.path.insert(0, _p)

import numpy as np
import ml_dtypes

from concourse import bacc, mybir, tile
from concourse.bass import IndirectOffsetOnAxis
from concourse.bass_utils import run_bass_kernel_spmd

F32 = mybir.dt.float32
BF16 = mybir.dt.float16    # 16-bit compute dtype: fp16 (10-bit mantissa)
I32 = mybir.dt.int32
AF = mybir.ActivationFunctionType
OP = mybir.AluOpType
AX = mybir.AxisListType


class Cfg:
    def __init__(self, B=32, S=512, W=15, E=300, H=256, C=2, n_cores=8,
                 use_cc=True, use_gather=True, use_ttr=False, repeat=1):
        self.B, self.S, self.W, self.E, self.H, self.C = B, S, W, E, H, C
        self.n_cores = n_cores
        self.use_cc = use_cc
        self.use_gather = use_gather
        self.use_ttr = use_ttr
        self.repeat = repeat
        self.G = 3 * H
        self.BC = B // n_cores                      # batch rows per core
        seg = S + 2 * (W - 1)                       # valid token cols per row
        self.SEG = ((self.BC * seg + 127) // 128 * 128) // self.BC \
            if (self.BC * seg) % 128 else seg
        # pad per-row segment so BC*SEG is a multiple of 128
        while (self.BC * self.SEG) % 128:
            self.SEG += 1
        assert self.SEG >= seg
        self.TC = self.BC * self.SEG                # token cols per core
        self.NT = self.TC // 128                    # gather tiles
        self.NR = self.BC * S                       # window rows per core
        self.HK = (H + 127) // 128                  # H partition tiles (2)
        self.GS = self.G // 128                     # G subtiles (6)
        self.EK = [(k * 128, min(128, E - k * 128))
                   for k in range((E + 127) // 128)]
        self.CHT = (2 * H) // 128                   # hidden channel tiles (4)
        # xg column chunks for the precompute matmuls
        self.XCH = [(i * 512, min(512, self.TC - i * 512))
                    for i in range((self.TC + 511) // 512)]
        assert H % 128 == 0 and self.G % 128 == 0


def build(cfg: Cfg):
    """Build + bacc-compile the Bass program. Returns (nc, out_name)."""
    nc = bacc.Bacc("TRN2", target_bir_lowering=False, debug=False,
                   enable_asserts=False, num_devices=cfg.n_cores)
    # register the BN epsilon as a const AP (0.0/1.0 are pre-registered)
    _eps_t = nc.alloc_sbuf_tensor("const-eps", [128, 1], F32)
    nc.gpsimd.memset(_eps_t.ap(), 1e-5)
    nc.const_aps.aps[(F32, 1e-5)] = _eps_t.ap()
    nc.all_engine_barrier()
    B, S, W, E, H, C = cfg.B, cfg.S, cfg.W, cfg.E, cfg.H, cfg.C
    BC, SEG, TC, NT, NR, HK, GS = (cfg.BC, cfg.SEG, cfg.TC, cfg.NT, cfg.NR,
                                   cfg.HK, cfg.GS)

    def din(name, shape, dt):
        return nc.dram_tensor(name, shape, dt, kind="ExternalInput").ap()

    ptab = din("ptab", [TC, E], F32)
    ids = din("ids", [TC, 1], I32)
    maskin = din("maskin", [128, NR], F32)
    wih = [din(f"wih{d}", [E, cfg.G], BF16) for d in range(2)]
    whh = [din(f"whh{d}", [H, cfg.G], BF16) for d in range(2)]
    bgd = [din(f"bg{d}", [128, GS], F32) for d in range(2)]
    bhnd = [din(f"bhn{d}", [128, HK], F32) for d in range(2)]
    identf_d = din("identf", [128, 128], F32)
    identb_d = din("identb", [128, 128], BF16)
    bng_d = din("bng", [128, cfg.CHT], F32)
    bnb_d = din("bnb", [128, cfg.CHT], F32)
    mb65_d = din("mb65", [128, cfg.CHT], F32)
    mwt_d = din("mwt", [2 * H, 2 * H], BF16)
    lwt_d = din("lwt", [2 * H, C], F32)
    lb4_d = din("lb4", [BC, C], F32)
    out_d = nc.dram_tensor("out", [BC, C], F32, kind="ExternalOutput").ap()

    inv_n = 1.0 / float(B * S)

    with tile.TileContext(nc) as tc:
        # ---- persistent constants -------------------------------------
        constp = tc.alloc_tile_pool(name="const", bufs=1)
        identf = constp.tile([128, 128], F32)
        identb = constp.tile([128, 128], BF16)
        nc.sync.dma_start(identf[:], identf_d[:])
        nc.sync.dma_start(identb[:], identb_d[:])
        whh_t = [[constp.tile([128, cfg.G], BF16, name=f"whh{d}_{k}")
                  for k in range(HK)] for d in range(2)]
        bg_t = [constp.tile([128, GS], F32, name=f"bg{d}") for d in range(2)]
        bhn_t = [constp.tile([128, HK], F32, name=f"bhn{d}") for d in range(2)]
        for d in range(2):
            for k in range(HK):
                nc.sync.dma_start(whh_t[d][k][:], whh[d][k * 128:(k + 1) * 128, :])
            nc.sync.dma_start(bg_t[d][:], bgd[d][:])
            nc.sync.dma_start(bhn_t[d][:], bhnd[d][:])

        # xg + h + hidden persistent tiles
        xgp = tc.alloc_tile_pool(name="xg", bufs=1)
        xg = [[xgp.tile([128, TC], BF16, name=f"xg{d}_{g}")
               for g in range(GS)] for d in range(2)]
        # 1-column-left-shifted copies of the n-gate xg so the tn add is
        # always 4B-aligned (fp16 2x DVE mode) regardless of step parity
        xgsh = [[xgp.tile([128, TC], BF16, name=f"xgsh{d}_{k}")
                 for k in range(HK)] for d in range(2)]
        hp = tc.alloc_tile_pool(name="h", bufs=1)
        h_t = [[hp.tile([128, NR], BF16, name=f"h{d}_{k}") for k in range(HK)]
               for d in range(2)]
        hidp = tc.alloc_tile_pool(name="hid", bufs=1, side="right")
        hid = [hidp.tile([128, NR], BF16, name=f"hid{ct}")
               for ct in range(cfg.CHT)]

        # optional hardware repeat loop for differential timing
        from contextlib import nullcontext
        rep_ctx = tc.For_i(0, cfg.repeat, 1) if cfg.repeat > 1 \
            else nullcontext()
        rep_ctx.__enter__()

        # ---- phase A: gather + transpose + xg precompute ---------------
        with tc.tile_pool(name="wihp", bufs=1) as wihp, \
             tc.tile_pool(name="idsp", bufs=2) as idsp, \
             tc.tile_pool(name="eraw", bufs=3) as erawp, \
             tc.tile_pool(name="eT", bufs=1) as eTp, \
             tc.tile_pool(name="tpsum", bufs=2, space="PSUM") as tpsump, \
             tc.tile_pool(name="xgpsum", bufs=4, space="PSUM") as xgpsump:
            wih_t = [[wihp.tile([128, cfg.G], BF16, name=f"wih{d}_{k}")
                      for k in range(len(cfg.EK))] for d in range(2)]
            for d in range(2):
                for k, (e0, ew) in enumerate(cfg.EK):
                    nc.sync.dma_start(wih_t[d][k][:ew, :], wih[d][e0:e0 + ew, :])
            eT = [eTp.tile([128, TC], BF16, name=f"eT{k}")
                  for k in range(len(cfg.EK))]
            for t in range(NT):
                idt = idsp.tile([128, 1], I32)
                nc.sync.dma_start(idt[:], ids[t * 128:(t + 1) * 128, :])
                er = erawp.tile([128, E], F32)
                if cfg.use_gather:
                    nc.gpsimd.indirect_dma_start(
                        out=er[:], out_offset=None, in_=ptab[:],
                        in_offset=IndirectOffsetOnAxis(ap=idt[:, :1], axis=0),
                    )
                else:
                    nc.sync.dma_start(er[:], ptab[t * 128:(t + 1) * 128, :])
                for k, (e0, ew) in enumerate(cfg.EK):
                    tp = tpsump.tile([128, 128], F32, space="PSUM")
                    nc.tensor.transpose(out=tp[:ew, :], in_=er[:, e0:e0 + ew],
                                        identity=identf[:])
                    nc.vector.tensor_copy(
                        out=eT[k][:ew, t * 128:(t + 1) * 128], in_=tp[:ew, :])
            for d in range(2):
                for g in range(GS):
                    for (c0, cw) in cfg.XCH:
                        p = xgpsump.tile([128, 512], F32, space="PSUM")
                        nk = len(cfg.EK)
                        for k, (e0, ew) in enumerate(cfg.EK):
                            nc.tensor.matmul(
                                p[:, :cw],
                                lhsT=wih_t[d][k][:ew, g * 128:(g + 1) * 128],
                                rhs=eT[k][:ew, c0:c0 + cw],
                                start=(k == 0), stop=(k == nk - 1))
                        nc.scalar.activation(
                            out=xg[d][g][:, c0:c0 + cw], in_=p[:, :cw],
                            func=AF.Identity, bias=bg_t[d][:, g:g + 1])

        for d in range(2):
            for k in range(HK):
                nc.vector.tensor_copy(out=xgsh[d][k][:, 0:TC - 1],
                                      in_=xg[d][2 * HK + k][:, 1:TC])

        # ---- phase B: the windowed GRU scan ----------------------------
        with tc.tile_pool(name="rz", bufs=3) as rzp, \
             tc.tile_pool(name="tg", bufs=3) as tgp, \
             tc.tile_pool(name="tn", bufs=3) as tnp, \
             tc.tile_pool(name="ng", bufs=3) as ngp, \
             tc.tile_pool(name="dg", bufs=3) as dgp, \
             tc.tile_pool(name="eg", bufs=3) as egp, \
             tc.tile_pool(name="prz", bufs=2, space="PSUM") as przp, \
             tc.tile_pool(name="pn", bufs=4, space="PSUM") as pnp:
            for w in range(W):
                last = (w == W - 1)
                for c in range(BC):
                    hc = slice(c * S, (c + 1) * S)
                    for d in range(2):
                        off = w if d == 0 else 2 * (W - 1) - w
                        base = c * SEG + off

                        def xs(g):
                            return xg[d][g][:, base:base + S]

                        if w == 0:
                            # h0 = 0: h1 = (1-z)*n, n = tanh(xn + r*bhh_n)
                            for k in range(HK):
                                r_ = tgp.tile([128, S], BF16, tag="t")
                                nc.scalar.activation(r_[:], xs(k), AF.Sigmoid)
                                z_ = dgp.tile([128, S], BF16, tag="d")
                                nc.scalar.activation(z_[:], xs(HK + k),
                                                     AF.Sigmoid)
                                tn_ = tnp.tile([128, S], BF16, tag="tn")
                                nc.vector.scalar_tensor_tensor(
                                    out=tn_[:], in0=r_[:],
                                    scalar=bhn_t[d][:, k:k + 1],
                                    in1=xs(2 * HK + k),
                                    op0=OP.mult, op1=OP.add)
                                nn_ = ngp.tile([128, S], BF16, tag="n")
                                nc.scalar.activation(nn_[:], tn_[:], AF.Tanh,
                                                     scale=-1.0)
                                nc.vector.scalar_tensor_tensor(
                                    out=h_t[d][k][:, hc], in0=z_[:],
                                    scalar=1.0, in1=nn_[:],
                                    op0=OP.subtract, op1=OP.mult)
                            continue

                        prz = []
                        for k in range(HK):
                            pz = przp.tile([128, 2 * S], F32, space="PSUM",
                                           tag="prz")
                            prz.append(pz)
                            for half, g in ((0, k), (1, HK + k)):
                                dst = pz[:, half * S:(half + 1) * S]
                                for kk in range(HK):
                                    nc.tensor.matmul(
                                        dst,
                                        lhsT=whh_t[d][kk][:, g * 128:(g + 1) * 128],
                                        rhs=h_t[d][kk][:, hc],
                                        start=(kk == 0), stop=False)
                                nc.tensor.matmul(dst, lhsT=identb[:],
                                                 rhs=xs(g),
                                                 start=False, stop=True)
                        pn = []
                        for k in range(HK):
                            pk = pnp.tile([128, S], F32, space="PSUM", tag="pn")
                            pn.append(pk)
                            g = 2 * HK + k
                            for kk in range(HK):
                                nc.tensor.matmul(
                                    pk[:],
                                    lhsT=whh_t[d][kk][:, g * 128:(g + 1) * 128],
                                    rhs=h_t[d][kk][:, hc],
                                    start=(kk == 0), stop=(kk == HK - 1))
                        for k in range(HK):
                            rz_ = rzp.tile([128, 2 * S], BF16, tag="rz")
                            nc.scalar.activation(rz_[:], prz[k][:], AF.Sigmoid)
                            t_ = tgp.tile([128, S], BF16, tag="t")
                            nc.vector.scalar_tensor_tensor(
                                out=t_[:], in0=pn[k][:],
                                scalar=bhn_t[d][:, k:k + 1],
                                in1=rz_[:, 0:S], op0=OP.add, op1=OP.mult)
                            tn_ = tnp.tile([128, S], BF16, tag="tn")
                            if base % 2 == 0:
                                xn_ap = xg[d][2 * HK + k][:, base:base + S]
                            else:
                                xn_ap = xgsh[d][k][:, base - 1:base - 1 + S]
                            nc.vector.tensor_tensor(
                                out=tn_[:], in0=t_[:], in1=xn_ap, op=OP.add)
                            n_ = ngp.tile([128, S], BF16, tag="n")
                            nc.scalar.activation(n_[:], tn_[:], AF.Tanh)
                            d_ = dgp.tile([128, S], BF16, tag="d")
                            nc.vector.tensor_tensor(
                                out=d_[:], in0=h_t[d][k][:, hc], in1=n_[:],
                                op=OP.subtract)
                            e_ = egp.tile([128, S], BF16, tag="e")
                            nc.vector.tensor_tensor(
                                out=e_[:], in0=rz_[:, S:2 * S], in1=d_[:],
                                op=OP.mult)
                            dest = (hid[d * HK + k] if last else h_t[d][k])
                            nc.vector.tensor_tensor(
                                out=dest[:, hc], in0=n_[:], in1=e_[:],
                                op=OP.add)

        if cfg.repeat == 1:
            hp.release()
            xgp.release()

        # ---- phase C: BN stats + AllReduce + affine --------------------
        nrmp = tc.alloc_tile_pool(name="nrm", bufs=1, side="right")
        nrm = [nrmp.tile([128, NR], BF16, name=f"nrm{ct}")
               for ct in range(cfg.CHT)]
        maskp = tc.alloc_tile_pool(name="maskp", bufs=1, side="right")
        mask_t = maskp.tile([128, NR], F32)
        nc.sync.dma_start(mask_t[:], maskin[:])
        with tc.tile_pool(name="scr", bufs=2) as scrp, \
             tc.tile_pool(name="stat", bufs=1) as statp, \
             tc.tile_pool(name="dram", bufs=1, space="DRAM") as dramp:
            sums = statp.tile([128, 2 * cfg.CHT], F32)
            dummy = statp.tile([128, 1], F32)
            for ct in range(cfg.CHT):
                sc = scrp.tile([128, NR], BF16, tag="scr")
                if cfg.use_ttr:
                    nc.vector.tensor_tensor_reduce(
                        out=sc[:], in0=hid[ct][:], in1=mask_t[:],
                        scale=1.0, scalar=0.0, op0=OP.mult, op1=OP.add,
                        accum_out=sums[:, ct:ct + 1])
                    nc.vector.tensor_tensor_reduce(
                        out=dummy.broadcast_to([128, NR]), in0=sc[:],
                        in1=sc[:], scale=1.0, scalar=0.0, op0=OP.mult,
                        op1=OP.add,
                        accum_out=sums[:, cfg.CHT + ct:cfg.CHT + ct + 1])
                else:
                    nc.vector.tensor_tensor(out=sc[:], in0=hid[ct][:],
                                            in1=mask_t[:], op=OP.mult)
                    nc.vector.tensor_reduce(out=sums[:, ct:ct + 1],
                                            in_=sc[:], axis=AX.X, op=OP.add)
                    sq = scrp.tile([128, NR], BF16, tag="scr2")
                    nc.vector.tensor_tensor(out=sq[:], in0=sc[:], in1=sc[:],
                                            op=OP.mult)
                    nc.vector.tensor_reduce(
                        out=sums[:, cfg.CHT + ct:cfg.CHT + ct + 1],
                        in_=sq[:], axis=AX.X, op=OP.add)
            gsums = statp.tile([128, 2 * cfg.CHT], F32)
            if cfg.use_cc:
                bnc_in = dramp.tile([128, 2 * cfg.CHT], F32)
                bnc_out = dramp.tile([128, 2 * cfg.CHT], F32,
                                     addr_space="Shared")
                nc.gpsimd.dma_start(bnc_in[:], sums[:])
                nc.gpsimd.collective_compute(
                    "AllReduce", OP.add,
                    replica_groups=[list(range(cfg.n_cores))],
                    ins=[bnc_in.opt()], outs=[bnc_out.opt()])
                nc.gpsimd.dma_start(gsums[:], bnc_out[:])
            else:
                nc.vector.tensor_copy(out=gsums[:], in_=sums[:])

            bng_t = statp.tile([128, cfg.CHT], F32)
            bnb_t = statp.tile([128, cfg.CHT], F32)
            nc.sync.dma_start(bng_t[:], bng_d[:])
            nc.sync.dma_start(bnb_t[:], bnb_d[:])
            abuf = statp.tile([128, cfg.CHT], F32)
            bbuf = statp.tile([128, cfg.CHT], F32)
            with nc.allow_low_precision("bn 1/sqrt + NR refine"), \
                 tc.tile_pool(name="stt", bufs=2) as sttp:
                for ct in range(cfg.CHT):
                    gs_s = gsums[:, ct:ct + 1]
                    gs_q = gsums[:, cfg.CHT + ct:cfg.CHT + ct + 1]
                    mu = sttp.tile([128, 1], F32, tag="mu")
                    nc.scalar.mul(mu[:], gs_s, inv_n)
                    mq = sttp.tile([128, 1], F32, tag="mq")
                    nc.scalar.square(mq[:], mu[:])
                    varp = sttp.tile([128, 1], F32, tag="var")
                    # var + eps = sumsq/n - mu^2 + eps
                    nc.vector.scalar_tensor_tensor(
                        out=varp[:], in0=gs_q, scalar=inv_n, in1=mq[:],
                        op0=OP.mult, op1=OP.subtract)
                    nc.scalar.add(varp[:], varp[:], 1e-5)  # varp = var + eps
                    sd = sttp.tile([128, 1], F32, tag="sd")
                    nc.scalar.sqrt(sd[:], varp[:])
                    y0 = sttp.tile([128, 1], F32, tag="y0")
                    nc.vector.reciprocal(y0[:], sd[:])
                    # one Newton step: y1 = y0*(1.5 - 0.5*var*y0^2)
                    y2 = sttp.tile([128, 1], F32, tag="y2")
                    nc.vector.tensor_tensor(out=y2[:], in0=y0[:], in1=y0[:],
                                            op=OP.mult)
                    vy2 = sttp.tile([128, 1], F32, tag="vy2")
                    nc.vector.tensor_tensor(out=vy2[:], in0=varp[:], in1=y2[:],
                                            op=OP.mult)
                    nc.vector.tensor_scalar(
                        out=vy2[:], in0=vy2[:], scalar1=-0.5, scalar2=1.5,
                        op0=OP.mult, op1=OP.add)
                    y1 = sttp.tile([128, 1], F32, tag="y1")
                    nc.vector.tensor_tensor(out=y1[:], in0=y0[:], in1=vy2[:],
                                            op=OP.mult)
                    nc.vector.tensor_tensor(out=abuf[:, ct:ct + 1],
                                            in0=bng_t[:, ct:ct + 1],
                                            in1=y1[:], op=OP.mult)
                    mua = sttp.tile([128, 1], F32, tag="mua")
                    nc.vector.tensor_tensor(out=mua[:], in0=mu[:],
                                            in1=abuf[:, ct:ct + 1],
                                            op=OP.mult)
                    nc.vector.tensor_tensor(out=bbuf[:, ct:ct + 1],
                                            in0=bnb_t[:, ct:ct + 1],
                                            in1=mua[:], op=OP.subtract)
            for ct in range(cfg.CHT):
                nc.vector.tensor_scalar(
                    out=nrm[ct][:], in0=hid[ct][:],
                    scalar1=abuf[:, ct:ct + 1], scalar2=bbuf[:, ct:ct + 1],
                    op0=OP.mult, op1=OP.add)
        # ---- phase D: MLP + masked max-pool + linear -------------------
        with tc.tile_pool(name="mwtp", bufs=1) as mwtp, \
             tc.tile_pool(name="tailc", bufs=1) as tailc, \
             tc.tile_pool(name="qp", bufs=3) as qp, \
             tc.tile_pool(name="pmlp", bufs=4, space="PSUM") as pmlpp, \
             tc.tile_pool(name="pfin", bufs=1, space="PSUM") as pfinp:
            mwt_t = [mwtp.tile([128, 2 * H], BF16, name=f"mwt{kt}")
                     for kt in range(cfg.CHT)]
            for kt in range(cfg.CHT):
                nc.sync.dma_start(mwt_t[kt][:], mwt_d[kt * 128:(kt + 1) * 128, :])
            mb65_t = tailc.tile([128, cfg.CHT], F32)
            nc.sync.dma_start(mb65_t[:], mb65_d[:])
            # moff = (mask - 1) * 65500 : 0 at live positions, -65500 at
            # masked ones; added after the mask-multiply so live values keep
            # full fp32 precision (a +65500 shift would quantize them).
            moff_t = tailc.tile([128, NR], F32)
            nc.vector.tensor_scalar(
                out=moff_t[:], in0=mask_t[:], scalar1=1.0, scalar2=65500.0,
                op0=OP.subtract, op1=OP.mult)
            lwt_t = [tailc.tile([128, C], F32, name=f"lwt{kt}")
                     for kt in range(cfg.CHT)]
            for kt in range(cfg.CHT):
                nc.sync.dma_start(lwt_t[kt][:], lwt_d[kt * 128:(kt + 1) * 128, :])
            lb_t = tailc.tile([128, C], F32)
            nc.sync.dma_start(lb_t[:BC, :], lb4_d[:, :])
            pld = [tailc.tile([128, BC], F32, name=f"pld{mt}")
                   for mt in range(cfg.CHT)]
            for c in range(BC):
                hc = slice(c * S, (c + 1) * S)
                for mt in range(cfg.CHT):
                    pm = pmlpp.tile([128, S], F32, space="PSUM", tag="pm")
                    for kt in range(cfg.CHT):
                        nc.tensor.matmul(
                            pm[:],
                            lhsT=mwt_t[kt][:, mt * 128:(mt + 1) * 128],
                            rhs=nrm[kt][:, hc],
                            start=(kt == 0), stop=(kt == cfg.CHT - 1))
                    qa = qp.tile([128, S], F32, tag="qa")
                    nc.vector.scalar_tensor_tensor(
                        out=qa[:], in0=pm[:], scalar=mb65_t[:, mt:mt + 1],
                        in1=mask_t[:, hc], op0=OP.add, op1=OP.mult)
                    q = qp.tile([128, S], F32, tag="q")
                    nc.vector.tensor_tensor(
                        out=q[:], in0=qa[:], in1=moff_t[:, hc], op=OP.add)
                    nc.vector.tensor_reduce(
                        out=pld[mt][:, c:c + 1], in_=q[:], axis=AX.X,
                        op=OP.max)
            pf = pfinp.tile([128, C], F32, space="PSUM")
            for mt in range(cfg.CHT):
                nc.tensor.matmul(pf[:BC, :], lhsT=pld[mt][:, :BC],
                                 rhs=lwt_t[mt][:, :],
                                 start=(mt == 0), stop=(mt == cfg.CHT - 1))
            ob = tailc.tile([128, C], F32)
            nc.vector.tensor_tensor(out=ob[:BC, :], in0=pf[:BC, :],
                                    in1=lb_t[:BC, :], op=OP.add)
            nc.sync.dma_start(out_d[:, :], ob[:BC, :])
        maskp.release()
        nrmp.release()
        rep_ctx.__exit__(None, None, None)
        if cfg.repeat > 1:
            hp.release()
            xgp.release()
        hidp.release()
        constp.release()

    nc.compile()
    return nc


def prep_inputs(inputs, cfg: Cfg):
    """Host-side sharding/prep. Returns in_maps (one dict per core)."""
    B, S, W, E, H, C = cfg.B, cfg.S, cfg.W, cfg.E, cfg.H, cfg.C
    x = np.asarray(inputs["x"]).astype(np.int64)
    emb = np.asarray(inputs["emb"], dtype=np.float32)
    mask = (x > 0).astype(np.float32)                       # [B, S]

    def bf(a):
        return np.ascontiguousarray(np.asarray(a, np.float32)
                                    .astype(np.float16))

    def f32(a):
        return np.ascontiguousarray(np.asarray(a, dtype=np.float32))

    shared = {}
    for d, sfx in enumerate("fb"):
        W_ih = np.asarray(inputs[f"W_ih_{sfx}"], np.float32)
        W_hh = np.asarray(inputs[f"W_hh_{sfx}"], np.float32)
        b_ih = np.asarray(inputs[f"b_ih_{sfx}"], np.float32)
        b_hh = np.asarray(inputs[f"b_hh_{sfx}"], np.float32)
        shared[f"wih{d}"] = bf(W_ih.T)                       # [E, G]
        shared[f"whh{d}"] = bf(W_hh.T)                       # [H, G]
        bfold = b_ih.copy()
        bfold[:2 * H] += b_hh[:2 * H]                        # r,z gates
        shared[f"bg{d}"] = f32(bfold.reshape(cfg.GS, 128).T)  # [128, GS]
        shared[f"bhn{d}"] = f32(b_hh[2 * H:].reshape(cfg.HK, 128).T)
    shared["identf"] = f32(np.eye(128))
    shared["identb"] = bf(np.eye(128))
    shared["bng"] = f32(np.asarray(inputs["bn_gamma"], np.float32)
                        .reshape(cfg.CHT, 128).T)
    shared["bnb"] = f32(np.asarray(inputs["bn_beta"], np.float32)
                        .reshape(cfg.CHT, 128).T)
    mlp_b = np.asarray(inputs["mlp_b"], np.float32)
    shared["mb65"] = f32(mlp_b.reshape(cfg.CHT, 128).T)
    shared["mwt"] = bf(np.asarray(inputs["mlp_W"], np.float32).T)
    lin_W = np.asarray(inputs["lin_W"], np.float32)
    lin_b = np.asarray(inputs["lin_b"], np.float32)
    shared["lwt"] = f32(lin_W.T)                             # [2H, C]
    shared["lb4"] = f32(np.broadcast_to(lin_b[None, :], (cfg.BC, C)))

    in_maps = []
    for core in range(cfg.n_cores):
        rows = x[core * cfg.BC:(core + 1) * cfg.BC]          # [BC, S]
        ids = np.zeros((cfg.BC, cfg.SEG), np.int64)
        ids[:, W - 1:W - 1 + S] = rows
        ids = ids.reshape(-1)                                # [TC]
        uids, inv = np.unique(ids, return_inverse=True)
        pt = np.zeros((cfg.TC, E), np.float32)
        if cfg.use_gather:
            pt[:len(uids)] = emb[uids]
        else:
            pt[:] = emb[ids]
        m = {k: v for k, v in shared.items()}
        m["ptab"] = pt
        m["ids"] = np.ascontiguousarray(inv.astype(np.int32)[:, None])
        mrow = mask[core * cfg.BC:(core + 1) * cfg.BC].reshape(-1)  # [NR]
        m["maskin"] = np.ascontiguousarray(
            np.broadcast_to(mrow[None, :], (128, cfg.NR)).astype(np.float32))
        in_maps.append(m)
    return in_maps


_CACHE = {}


def get_compiled(cfg: Cfg | None = None):
    key = "default" if cfg is None else id(cfg)
    if key not in _CACHE:
        _CACHE[key] = build(cfg or Cfg())
    return _CACHE[key]


def kernel(**inputs) -> np.ndarray:
    cfg = Cfg()
    nc = get_compiled(None)
    in_maps = prep_inputs(inputs, cfg)
    res = run_bass_kernel_spmd(nc, in_maps, core_ids=list(range(cfg.n_cores)))
    return np.concatenate([res.results[i]["out"] for i in range(cfg.n_cores)],
                          axis=0).astype(np.float32)


# revision 19
# speedup vs baseline: 1135.5782x; 1090.1791x over previous
"""Trainium2 Bass kernel for nn_DRNN_75204877353433.

Windowed bidirectional GRU (W=15) over [B=32, S=512] token ids ->
batch-norm (training stats over B,S) -> MLP -> masked max-pool -> linear.

Strategy (8 NeuronCores, data-parallel over batch):
  - each core handles BC=4 batch rows (2048 window positions), runs the
    full windowed GRU scan for both directions;
  - embedding table is pruned per core on host (only rows the core's
    tokens index, a sharding of the table); the gather itself runs on
    device via indirect DMA;
  - per-token input projections xg = W_ih @ e(tok) + biases are computed
    once per direction on the PE (15x reuse across window positions);
  - recurrent steps: hg = W_hh @ h accumulated in PSUM, with the xr/xz
    additions folded in as identity matmuls; sigmoids/tanh on the scalar
    engine read PSUM directly; the n-gate r*hn and the h-update run on
    the vector engine in bf16;
  - batch-norm statistics: fused multiply+reduce passes, 4KB AllReduce
    across the 8 cores;
  - mask is folded through the MLP algebraically; masked positions get
    their -65500 via a separate additive (mask-1)*65500 term so live
    values keep full fp32 precision.
"""

import sys

for _p in ("/opt/trn_rl_repo",):
    if _p not in sys.path:
        sys.path.insert(0, _p)

import numpy as np
import ml_dtypes

from concourse import bacc, mybir, tile
from concourse.bass import IndirectOffsetOnAxis
from concourse.bass_utils import run_bass_kernel_spmd

F32 = mybir.dt.float32
BF16 = mybir.dt.float16    # 16-bit compute dtype: fp16 (10-bit mantissa)
I32 = mybir.dt.int32
AF = mybir.ActivationFunctionType
OP = mybir.AluOpType
AX = mybir.AxisListType


class Cfg:
    def __init__(self, B=32, S=512, W=15, E=300, H=256, C=2, n_cores=8,
                 use_cc=True, use_gather=True, use_ttr=False, repeat=1):
        self.B, self.S, self.W, self.E, self.H, self.C = B, S, W, E, H, C
        self.n_cores = n_cores
        self.use_cc = use_cc
        self.use_gather = use_gather
        self.use_ttr = use_ttr
        self.repeat = repeat
        self.G = 3 * H
        self.BC = B // n_cores                      # batch rows per core
        seg = S + 2 * (W - 1)                       # valid token cols per row
        self.SEG = ((self.BC * seg + 127) // 128 * 128) // self.BC \
            if (self.BC * seg) % 128 else seg
        # pad per-row segment so BC*SEG is a multiple of 128
        while (self.BC * self.SEG) % 128:
            self.SEG += 1
        assert self.SEG >= seg
        self.TC = self.BC * self.SEG                # token cols per core
        self.NT = self.TC // 128                    # gather tiles
        self.NR = self.BC * S                       # window rows per core
        self.HK = (H + 127) // 128                  # H partition tiles (2)
        self.GS = self.G // 128                     # G subtiles (6)
        self.EK = [(k * 128, min(128, E - k * 128))
                   for k in range((E + 127) // 128)]
        self.CHT = (2 * H) // 128                   # hidden channel tiles (4)
        # xg column chunks for the precompute matmuls
        self.XCH = [(i * 512, min(512, self.TC - i * 512))
                    for i in range((self.TC + 511) // 512)]
        assert H % 128 == 0 and self.G % 128 == 0


def build(cfg: Cfg):
    """Build + bacc-compile the Bass program. Returns (nc, out_name)."""
    nc = bacc.Bacc("TRN2", target_bir_lowering=False, debug=False,
                   enable_asserts=False, num_devices=cfg.n_cores)
    # register the BN epsilon as a const AP (0.0/1.0 are pre-registered)
    _eps_t = nc.alloc_sbuf_tensor("const-eps", [128, 1], F32)
    nc.gpsimd.memset(_eps_t.ap(), 1e-5)
    nc.const_aps.aps[(F32, 1e-5)] = _eps_t.ap()
    nc.all_engine_barrier()
    B, S, W, E, H, C = cfg.B, cfg.S, cfg.W, cfg.E, cfg.H, cfg.C
    BC, SEG, TC, NT, NR, HK, GS = (cfg.BC, cfg.SEG, cfg.TC, cfg.NT, cfg.NR,
                                   cfg.HK, cfg.GS)

    def din(name, shape, dt):
        return nc.dram_tensor(name, shape, dt, kind="ExternalInput").ap()

    ptab = din("ptab", [TC, E], F32)
    ids = din("ids", [TC, 1], I32)
    maskin = din("maskin", [128, NR], F32)
    wih = [din(f"wih{d}", [E, cfg.G], BF16) for d in range(2)]
    whh = [din(f"whh{d}", [H, cfg.G], BF16) for d in range(2)]
    bgd = [din(f"bg{d}", [128, GS], F32) for d in range(2)]
    bhnd = [din(f"bhn{d}", [128, HK], F32) for d in range(2)]
    identf_d = din("identf", [128, 128], F32)
    identb_d = din("identb", [128, 128], BF16)
    bng_d = din("bng", [128, cfg.CHT], F32)
    bnb_d = din("bnb", [128, cfg.CHT], F32)
    mb65_d = din("mb65", [128, cfg.CHT], F32)
    mwt_d = din("mwt", [2 * H, 2 * H], BF16)
    lwt_d = din("lwt", [2 * H, C], F32)
    lb4_d = din("lb4", [BC, C], F32)
    out_d = nc.dram_tensor("out", [BC, C], F32, kind="ExternalOutput").ap()

    inv_n = 1.0 / float(B * S)

    with tile.TileContext(nc) as tc:
        # ---- persistent constants -------------------------------------
        constp = tc.alloc_tile_pool(name="const", bufs=1)
        identf = constp.tile([128, 128], F32)
        identb = constp.tile([128, 128], BF16)
        nc.sync.dma_start(identf[:], identf_d[:])
        nc.sync.dma_start(identb[:], identb_d[:])
        whh_t = [[constp.tile([128, cfg.G], BF16, name=f"whh{d}_{k}")
                  for k in range(HK)] for d in range(2)]
        bg_t = [constp.tile([128, GS], F32, name=f"bg{d}") for d in range(2)]
        bhn_t = [constp.tile([128, HK], F32, name=f"bhn{d}") for d in range(2)]
        for d in range(2):
            for k in range(HK):
                nc.sync.dma_start(whh_t[d][k][:], whh[d][k * 128:(k + 1) * 128, :])
            nc.sync.dma_start(bg_t[d][:], bgd[d][:])
            nc.sync.dma_start(bhn_t[d][:], bhnd[d][:])

        # xg + h + hidden persistent tiles
        xgp = tc.alloc_tile_pool(name="xg", bufs=1)
        xg = [[xgp.tile([128, TC], BF16, name=f"xg{d}_{g}")
               for g in range(GS)] for d in range(2)]
        # 1-column-left-shifted copies of the n-gate xg so the tn add is
        # always 4B-aligned (fp16 2x DVE mode) regardless of step parity
        xgsh = [[xgp.tile([128, TC], BF16, name=f"xgsh{d}_{k}")
                 for k in range(HK)] for d in range(2)]
        hp = tc.alloc_tile_pool(name="h", bufs=1)
        h_t = [[hp.tile([128, NR], BF16, name=f"h{d}_{k}") for k in range(HK)]
               for d in range(2)]
        hidp = tc.alloc_tile_pool(name="hid", bufs=1, side="right")
        hid = [hidp.tile([128, NR], BF16, name=f"hid{ct}")
               for ct in range(cfg.CHT)]

        # optional hardware repeat loop for differential timing
        from contextlib import nullcontext
        rep_ctx = tc.For_i(0, cfg.repeat, 1) if cfg.repeat > 1 \
            else nullcontext()
        rep_ctx.__enter__()

        # ---- phase A: gather + transpose + xg precompute ---------------
        with tc.tile_pool(name="wihp", bufs=1) as wihp, \
             tc.tile_pool(name="idsp", bufs=2) as idsp, \
             tc.tile_pool(name="eraw", bufs=3) as erawp, \
             tc.tile_pool(name="eT", bufs=1) as eTp, \
             tc.tile_pool(name="tpsum", bufs=2, space="PSUM") as tpsump, \
             tc.tile_pool(name="xgpsum", bufs=4, space="PSUM") as xgpsump:
            wih_t = [[wihp.tile([128, cfg.G], BF16, name=f"wih{d}_{k}")
                      for k in range(len(cfg.EK))] for d in range(2)]
            for d in range(2):
                for k, (e0, ew) in enumerate(cfg.EK):
                    nc.sync.dma_start(wih_t[d][k][:ew, :], wih[d][e0:e0 + ew, :])
            eT = [eTp.tile([128, TC], BF16, name=f"eT{k}")
                  for k in range(len(cfg.EK))]
            for t in range(NT):
                idt = idsp.tile([128, 1], I32)
                nc.sync.dma_start(idt[:], ids[t * 128:(t + 1) * 128, :])
                er = erawp.tile([128, E], F32)
                if cfg.use_gather:
                    nc.gpsimd.indirect_dma_start(
                        out=er[:], out_offset=None, in_=ptab[:],
                        in_offset=IndirectOffsetOnAxis(ap=idt[:, :1], axis=0),
                    )
                else:
                    nc.sync.dma_start(er[:], ptab[t * 128:(t + 1) * 128, :])
                for k, (e0, ew) in enumerate(cfg.EK):
                    tp = tpsump.tile([128, 128], F32, space="PSUM")
                    nc.tensor.transpose(out=tp[:ew, :], in_=er[:, e0:e0 + ew],
                                        identity=identf[:])
                    nc.vector.tensor_copy(
                        out=eT[k][:ew, t * 128:(t + 1) * 128], in_=tp[:ew, :])
            for d in range(2):
                for g in range(GS):
                    for (c0, cw) in cfg.XCH:
                        p = xgpsump.tile([128, 512], F32, space="PSUM")
                        nk = len(cfg.EK)
                        for k, (e0, ew) in enumerate(cfg.EK):
                            nc.tensor.matmul(
                                p[:, :cw],
                                lhsT=wih_t[d][k][:ew, g * 128:(g + 1) * 128],
                                rhs=eT[k][:ew, c0:c0 + cw],
                                start=(k == 0), stop=(k == nk - 1))
                        nc.scalar.activation(
                            out=xg[d][g][:, c0:c0 + cw], in_=p[:, :cw],
                            func=AF.Identity, bias=bg_t[d][:, g:g + 1])

        for d in range(2):
            for k in range(HK):
                nc.vector.tensor_copy(out=xgsh[d][k][:, 0:TC - 1],
                                      in_=xg[d][2 * HK + k][:, 1:TC])

        # ---- phase B: the windowed GRU scan ----------------------------
        with tc.tile_pool(name="rz", bufs=3) as rzp, \
             tc.tile_pool(name="tg", bufs=3) as tgp, \
             tc.tile_pool(name="tn", bufs=3) as tnp, \
             tc.tile_pool(name="ng", bufs=3) as ngp, \
             tc.tile_pool(name="dg", bufs=3) as dgp, \
             tc.tile_pool(name="eg", bufs=3) as egp, \
             tc.tile_pool(name="prz", bufs=2, space="PSUM") as przp, \
             tc.tile_pool(name="pn", bufs=4, space="PSUM") as pnp:
            for w in range(W):
                last = (w == W - 1)
                for c in range(BC):
                    hc = slice(c * S, (c + 1) * S)
                    for d in range(2):
                        off = w if d == 0 else 2 * (W - 1) - w
                        base = c * SEG + off

                        def xs(g):
                            return xg[d][g][:, base:base + S]

                        if w == 0:
                            # h0 = 0: h1 = (1-z)*n, n = tanh(xn + r*bhh_n)
                            for k in range(HK):
                                r_ = tgp.tile([128, S], BF16, tag="t")
                                nc.scalar.activation(r_[:], xs(k), AF.Sigmoid)
                                z_ = dgp.tile([128, S], BF16, tag="d")
                                nc.scalar.activation(z_[:], xs(HK + k),
                                                     AF.Sigmoid)
                                tn_ = tnp.tile([128, S], BF16, tag="tn")
                                nc.vector.scalar_tensor_tensor(
                                    out=tn_[:], in0=r_[:],
                                    scalar=bhn_t[d][:, k:k + 1],
                                    in1=xs(2 * HK + k),
                                    op0=OP.mult, op1=OP.add)
                                nn_ = ngp.tile([128, S], BF16, tag="n")
                                nc.scalar.activation(nn_[:], tn_[:], AF.Tanh,
                                                     scale=-1.0)
                                nc.vector.scalar_tensor_tensor(
                                    out=h_t[d][k][:, hc], in0=z_[:],
                                    scalar=1.0, in1=nn_[:],
                                    op0=OP.subtract, op1=OP.mult)
                            continue

                        prz = []
                        for k in range(HK):
                            pz = przp.tile([128, 2 * S], F32, space="PSUM",
                                           tag="prz")
                            prz.append(pz)
                            for half, g in ((0, k), (1, HK + k)):
                                dst = pz[:, half * S:(half + 1) * S]
                                for kk in range(HK):
                                    nc.tensor.matmul(
                                        dst,
                                        lhsT=whh_t[d][kk][:, g * 128:(g + 1) * 128],
                                        rhs=h_t[d][kk][:, hc],
                                        start=(kk == 0), stop=False)
                                nc.tensor.matmul(dst, lhsT=identb[:],
                                                 rhs=xs(g),
                                                 start=False, stop=True)
                        pn = []
                        for k in range(HK):
                            pk = pnp.tile([128, S], F32, space="PSUM", tag="pn")
                            pn.append(pk)
                            g = 2 * HK + k
                            for kk in range(HK):
                                nc.tensor.matmul(
                                    pk[:],
                                    lhsT=whh_t[d][kk][:, g * 128:(g + 1) * 128],
                                    rhs=h_t[d][kk][:, hc],
                                    start=(kk == 0), stop=(kk == HK - 1))
                        for k in range(HK):
                            rz_ = rzp.tile([128, 2 * S], BF16, tag="rz")
                            nc.scalar.activation(rz_[:], prz[k][:], AF.Sigmoid)
                            t_ = tgp.tile([128, S], BF16, tag="t")
                            nc.vector.scalar_tensor_tensor(
                                out=t_[:], in0=pn[k][:],
                                scalar=bhn_t[d][:, k:k + 1],
                                in1=rz_[:, 0:S], op0=OP.add, op1=OP.mult)
                            tn_ = tnp.tile([128, S], BF16, tag="tn")
                            if base % 2 == 0:
                                xn_ap = xg[d][2 * HK + k][:, base:base + S]
                            else:
                                xn_ap = xgsh[d][k][:, base - 1:base - 1 + S]
                            nc.vector.tensor_tensor(
                                out=tn_[:], in0=t_[:], in1=xn_ap, op=OP.add)
                            n_ = ngp.tile([128, S], BF16, tag="n")
                            nc.scalar.activation(n_[:], tn_[:], AF.Tanh)
                            d_ = dgp.tile([128, S], BF16, tag="d")
                            nc.vector.tensor_tensor(
                                out=d_[:], in0=h_t[d][k][:, hc], in1=n_[:],
                                op=OP.subtract)
                            e_ = egp.tile([128, S], BF16, tag="e")
                            nc.vector.tensor_tensor(
                                out=e_[:], in0=rz_[:, S:2 * S], in1=d_[:],
                                op=OP.mult)
                            dest = (hid[d * HK + k] if last else h_t[d][k])
                            nc.vector.tensor_tensor(
                                out=dest[:, hc], in0=n_[:], in1=e_[:],
                                op=OP.add)

        if cfg.repeat == 1:
            hp.release()
            xgp.release()

        # ---- phase C: BN stats + AllReduce + affine --------------------
        nrmp = tc.alloc_tile_pool(name="nrm", bufs=1, side="right")
        nrm = [nrmp.tile([128, NR], BF16, name=f"nrm{ct}")
               for ct in range(cfg.CHT)]
        maskp = tc.alloc_tile_pool(name="maskp", bufs=1, side="right")
        mask_t = maskp.tile([128, NR], F32)
        nc.sync.dma_start(mask_t[:], maskin[:])
        with tc.tile_pool(name="scr", bufs=2) as scrp, \
             tc.tile_pool(name="stat", bufs=1) as statp, \
             tc.tile_pool(name="dram", bufs=1, space="DRAM") as dramp:
            sums = statp.tile([128, 2 * cfg.CHT], F32)
            dummy = statp.tile([128, 1], F32)
            for ct in range(cfg.CHT):
                sc = scrp.tile([128, NR], BF16, tag="scr")
                if cfg.use_ttr:
                    nc.vector.tensor_tensor_reduce(
                        out=sc[:], in0=hid[ct][:], in1=mask_t[:],
                        scale=1.0, scalar=0.0, op0=OP.mult, op1=OP.add,
                        accum_out=sums[:, ct:ct + 1])
                    nc.vector.tensor_tensor_reduce(
                        out=dummy.broadcast_to([128, NR]), in0=sc[:],
                        in1=sc[:], scale=1.0, scalar=0.0, op0=OP.mult,
                        op1=OP.add,
                        accum_out=sums[:, cfg.CHT + ct:cfg.CHT + ct + 1])
                else:
                    nc.vector.tensor_tensor(out=sc[:], in0=hid[ct][:],
                                            in1=mask_t[:], op=OP.mult)
                    nc.vector.tensor_reduce(out=sums[:, ct:ct + 1],
                                            in_=sc[:], axis=AX.X, op=OP.add)
                    sq = scrp.tile([128, NR], BF16, tag="scr2")
                    nc.vector.tensor_tensor(out=sq[:], in0=sc[:], in1=sc[:],
                                            op=OP.mult)
                    nc.vector.tensor_reduce(
                        out=sums[:, cfg.CHT + ct:cfg.CHT + ct + 1],
                        in_=sq[:], axis=AX.X, op=OP.add)
            gsums = statp.tile([128, 2 * cfg.CHT], F32)
            if cfg.use_cc:
                bnc_in = dramp.tile([128, 2 * cfg.CHT], F32)
                bnc_out = dramp.tile([128, 2 * cfg.CHT], F32,
                                     addr_space="Shared")
                nc.gpsimd.dma_start(bnc_in[:], sums[:])
                nc.gpsimd.collective_compute(
                    "AllReduce", OP.add,
                    replica_groups=[list(range(cfg.n_cores))],
                    ins=[bnc_in.opt()], outs=[bnc_out.opt()])
                nc.gpsimd.dma_start(gsums[:], bnc_out[:])
            else:
                nc.vector.tensor_copy(out=gsums[:], in_=sums[:])

            bng_t = statp.tile([128, cfg.CHT], F32)
            bnb_t = statp.tile([128, cfg.CHT], F32)
            nc.sync.dma_start(bng_t[:], bng_d[:])
            nc.sync.dma_start(bnb_t[:], bnb_d[:])
            abuf = statp.tile([128, cfg.CHT], F32)
            bbuf = statp.tile([128, cfg.CHT], F32)
            with nc.allow_low_precision("bn 1/sqrt + NR refine"), \
                 tc.tile_pool(name="stt", bufs=2) as sttp:
                for ct in range(cfg.CHT):
                    gs_s = gsums[:, ct:ct + 1]
                    gs_q = gsums[:, cfg.CHT + ct:cfg.CHT + ct + 1]
                    mu = sttp.tile([128, 1], F32, tag="mu")
                    nc.scalar.mul(mu[:], gs_s, inv_n)
                    mq = sttp.tile([128, 1], F32, tag="mq")
                    nc.scalar.square(mq[:], mu[:])
                    varp = sttp.tile([128, 1], F32, tag="var")
                    # var + eps = sumsq/n - mu^2 + eps
                    nc.vector.scalar_tensor_tensor(
                        out=varp[:], in0=gs_q, scalar=inv_n, in1=mq[:],
                        op0=OP.mult, op1=OP.subtract)
                    nc.scalar.add(varp[:], varp[:], 1e-5)  # varp = var + eps
                    sd = sttp.tile([128, 1], F32, tag="sd")
                    nc.scalar.sqrt(sd[:], varp[:])
                    y0 = sttp.tile([128, 1], F32, tag="y0")
                    nc.vector.reciprocal(y0[:], sd[:])
                    # one Newton step: y1 = y0*(1.5 - 0.5*var*y0^2)
                    y2 = sttp.tile([128, 1], F32, tag="y2")
                    nc.vector.tensor_tensor(out=y2[:], in0=y0[:], in1=y0[:],
                                            op=OP.mult)
                    vy2 = sttp.tile([128, 1], F32, tag="vy2")
                    nc.vector.tensor_tensor(out=vy2[:], in0=varp[:], in1=y2[:],
                                            op=OP.mult)
                    nc.vector.tensor_scalar(
                        out=vy2[:], in0=vy2[:], scalar1=-0.5, scalar2=1.5,
                        op0=OP.mult, op1=OP.add)
                    y1 = sttp.tile([128, 1], F32, tag="y1")
                    nc.vector.tensor_tensor(out=y1[:], in0=y0[:], in1=vy2[:],
                                            op=OP.mult)
                    nc.vector.tensor_tensor(out=abuf[:, ct:ct + 1],
                                            in0=bng_t[:, ct:ct + 1],
                                            in1=y1[:], op=OP.mult)
                    mua = sttp.tile([128, 1], F32, tag="mua")
                    nc.vector.tensor_tensor(out=mua[:], in0=mu[:],
                                            in1=abuf[:, ct:ct + 1],
                                            op=OP.mult)
                    nc.vector.tensor_tensor(out=bbuf[:, ct:ct + 1],
                                            in0=bnb_t[:, ct:ct + 1],
                                            in1=mua[:], op=OP.subtract)
            for ct in range(cfg.CHT):
                nc.vector.tensor_scalar(
                    out=nrm[ct][:], in0=hid[ct][:],
                    scalar1=abuf[:, ct:ct + 1], scalar2=bbuf[:, ct:ct + 1],
                    op0=OP.mult, op1=OP.add)
        # ---- phase D: MLP + masked max-pool + linear -------------------
        with tc.tile_pool(name="mwtp", bufs=1) as mwtp, \
             tc.tile_pool(name="tailc", bufs=1) as tailc, \
             tc.tile_pool(name="qp", bufs=3) as qp, \
             tc.tile_pool(name="pmlp", bufs=4, space="PSUM") as pmlpp, \
             tc.tile_pool(name="pfin", bufs=1, space="PSUM") as pfinp:
            mwt_t = [mwtp.tile([128, 2 * H], BF16, name=f"mwt{kt}")
                     for kt in range(cfg.CHT)]
            for kt in range(cfg.CHT):
                nc.sync.dma_start(mwt_t[kt][:], mwt_d[kt * 128:(kt + 1) * 128, :])
            mb65_t = tailc.tile([128, cfg.CHT], F32)
            nc.sync.dma_start(mb65_t[:], mb65_d[:])
            # moff = (mask - 1) * 65500 : 0 at live positions, -65500 at
            # masked ones; added after the mask-multiply so live values keep
            # full fp32 precision (a +65500 shift would quantize them).
            moff_t = tailc.tile([128, NR], F32)
            nc.vector.tensor_scalar(
                out=moff_t[:], in0=mask_t[:], scalar1=1.0, scalar2=65500.0,
                op0=OP.subtract, op1=OP.mult)
            lwt_t = [tailc.tile([128, C], F32, name=f"lwt{kt}")
                     for kt in range(cfg.CHT)]
            for kt in range(cfg.CHT):
                nc.sync.dma_start(lwt_t[kt][:], lwt_d[kt * 128:(kt + 1) * 128, :])
            lb_t = tailc.tile([128, C], F32)
            nc.sync.dma_start(lb_t[:BC, :], lb4_d[:, :])
            pld = [tailc.tile([128, BC], F32, name=f"pld{mt}")
                   for mt in range(cfg.CHT)]
            for c in range(BC):
                hc = slice(c * S, (c + 1) * S)
                for mt in range(cfg.CHT):
                    pm = pmlpp.tile([128, S], F32, space="PSUM", tag="pm")
                    for kt in range(cfg.CHT):
                        nc.tensor.matmul(
                            pm[:],
                            lhsT=mwt_t[kt][:, mt * 128:(mt + 1) * 128],
                            rhs=nrm[kt][:, hc],
                            start=(kt == 0), stop=(kt == cfg.CHT - 1))
                    qa = qp.tile([128, S], F32, tag="qa")
                    nc.vector.scalar_tensor_tensor(
                        out=qa[:], in0=pm[:], scalar=mb65_t[:, mt:mt + 1],
                        in1=mask_t[:, hc], op0=OP.add, op1=OP.mult)
                    q = qp.tile([128, S], F32, tag="q")
                    nc.vector.tensor_tensor(
                        out=q[:], in0=qa[:], in1=moff_t[:, hc], op=OP.add)
                    nc.vector.tensor_reduce(
                        out=pld[mt][:, c:c + 1], in_=q[:], axis=AX.X,
                        op=OP.max)
            pf = pfinp.tile([128, C], F32, space="PSUM")
            for mt in range(cfg.CHT):
                nc.tensor.matmul(pf[:BC, :], lhsT=pld[mt][:, :BC],
                                 rhs=lwt_t[mt][:, :],
                                 start=(mt == 0), stop=(mt == cfg.CHT - 1))
            ob = tailc.tile([128, C], F32)
            nc.vector.tensor_tensor(out=ob[:BC, :], in0=pf[:BC, :],
                                    in1=lb_t[:BC, :], op=OP.add)
            nc.sync.dma_start(out_d[:, :], ob[:BC, :])
        maskp.release()
        nrmp.release()
        rep_ctx.__exit__(None, None, None)
        if cfg.repeat > 1:
            hp.release()
            xgp.release()
        hidp.release()
        constp.release()

    nc.compile()
    return nc


def prep_inputs(inputs, cfg: Cfg):
    """Host-side sharding/prep. Returns in_maps (one dict per core)."""
    B, S, W, E, H, C = cfg.B, cfg.S, cfg.W, cfg.E, cfg.H, cfg.C
    x = np.asarray(inputs["x"]).astype(np.int64)
    emb = np.asarray(inputs["emb"], dtype=np.float32)
    mask = (x > 0).astype(np.float32)                       # [B, S]

    def bf(a):
        return np.ascontiguousarray(np.asarray(a, np.float32)
                                    .astype(np.float16))

    def f32(a):
        return np.ascontiguousarray(np.asarray(a, dtype=np.float32))

    shared = {}
    for d, sfx in enumerate("fb"):
        W_ih = np.asarray(inputs[f"W_ih_{sfx}"], np.float32)
        W_hh = np.asarray(inputs[f"W_hh_{sfx}"], np.float32)
        b_ih = np.asarray(inputs[f"b_ih_{sfx}"], np.float32)
        b_hh = np.asarray(inputs[f"b_hh_{sfx}"], np.float32)
        shared[f"wih{d}"] = bf(W_ih.T)                       # [E, G]
        shared[f"whh{d}"] = bf(W_hh.T)                       # [H, G]
        bfold = b_ih.copy()
        bfold[:2 * H] += b_hh[:2 * H]                        # r,z gates
        shared[f"bg{d}"] = f32(bfold.reshape(cfg.GS, 128).T)  # [128, GS]
        shared[f"bhn{d}"] = f32(b_hh[2 * H:].reshape(cfg.HK, 128).T)
    shared["identf"] = f32(np.eye(128))
    shared["identb"] = bf(np.eye(128))
    shared["bng"] = f32(np.asarray(inputs["bn_gamma"], np.float32)
                        .reshape(cfg.CHT, 128).T)
    shared["bnb"] = f32(np.asarray(inputs["bn_beta"], np.float32)
                        .reshape(cfg.CHT, 128).T)
    mlp_b = np.asarray(inputs["mlp_b"], np.float32)
    shared["mb65"] = f32(mlp_b.reshape(cfg.CHT, 128).T)
    shared["mwt"] = bf(np.asarray(inputs["mlp_W"], np.float32).T)
    lin_W = np.asarray(inputs["lin_W"], np.float32)
    lin_b = np.asarray(inputs["lin_b"], np.float32)
    shared["lwt"] = f32(lin_W.T)                             # [2H, C]
    shared["lb4"] = f32(np.broadcast_to(lin_b[None, :], (cfg.BC, C)))

    in_maps = []
    for core in range(cfg.n_cores):
        rows = x[core * cfg.BC:(core + 1) * cfg.BC]          # [BC, S]
        ids = np.zeros((cfg.BC, cfg.SEG), np.int64)
        ids[:, W - 1:W - 1 + S] = rows
        ids = ids.reshape(-1)                                # [TC]
        uids, inv = np.unique(ids, return_inverse=True)
        pt = np.zeros((cfg.TC, E), np.float32)
        if cfg.use_gather:
            pt[:len(uids)] = emb[uids]
        else:
            pt[:] = emb[ids]
        m = {k: v for k, v in shared.items()}
        m["ptab"] = pt
        m["ids"] = np.ascontiguousarray(inv.astype(np.int32)[:, None])
        mrow = mask[core * cfg.BC:(core + 1) * cfg.BC].reshape(-1)  # [NR]
        m["maskin"] = np.ascontiguousarray(
            np.broadcast_to(mrow[None, :], (128, cfg.NR)).astype(np.float32))
        in_maps.append(m)
    return in_maps


_CACHE = {}


def get_compiled(cfg: Cfg | None = None):
    key = "default" if cfg is None else id(cfg)
    if key not in _CACHE:
        _CACHE[key] = build(cfg or Cfg())
    return _CACHE[key]


def kernel(**inputs) -> np.ndarray:
    cfg = Cfg()
    nc = get_compiled(None)
    in_maps = prep_inputs(inputs, cfg)
    res = run_bass_kernel_spmd(nc, in_maps, core_ids=list(range(cfg.n_cores)))
    return np.concatenate([res.results[i]["out"] for i in range(cfg.n_cores)],
                          axis=0).astype(np.float32)
